# revision 91
# baseline (speedup 1.0000x reference)
"""Llama attention layer on 8 TRN2 NeuronCores.

Sharding: core = (batch b in 0..1) x (head-group g in 0..3), 4 heads each.
Per core: full hidden_states[b] (transposed on host), per-head-packed
column slices of wq/wk, k-packed wv, row slice of wo.T. Host sums the 4
per-head-group o_proj partials per batch (bf16 partials, f32 sum).

Single fully software-pipelined program (issue order = engine queue order):

  qk(all heads, chunk0) + qk(h0/h1, c1-3) | V | att(0..2) cells, each
  interleaved 1:1 with a deferred projection chain | att(3) cells
  carrying o_proj chains of the previous chunk-group | o_proj tail

Attention cell (h,c): scoresT tiles [sk 128, sq 512] -> exp on ACT with
per-partition mask bias -> pa accumulation (attn_outT) on PE. The
softmax denominator costs no PE matmuls: a DVE bf16 add-tree over the
exp tiles feeds a GPSIMD partition_all_reduce; normalize = DVE
reciprocal + multiply, deferred into the next cell so the allreduce
round-trip never blocks the DVE queue. Interleaving keeps the PE issue
rate per exp above the ACT engine's ~570ns/tile service time, and the
eight chunk-0 projection chains borrow the idle attention PSUM pools at
startup so RoPE lag on the cos/sin DMA can't stall the PE.
"""

import numpy as np
import ml_dtypes

B, S, H, NH, HD = 2, 2048, 2048, 16, 128
G = 4            # heads per core
HG = G * HD      # 512 head-dim columns per core
KT = H // 128    # 16 contraction chunks
ST = S // 128    # 16 sequence tiles of 128
SC = S // 512    # 4 sequence chunks of 512
NCORES = 8

_NC_CACHE = {}


def _ensure_path():
    import sys
    for p in ('/opt/trn_rl_repo', '/opt/pypackages'):
        if p not in sys.path:
            sys.path.append(p)


def _build_nc():
    _ensure_path()
    from contextlib import ExitStack
    import concourse.tile as tile
    from concourse import bacc, mybir, bass_isa

    bf16 = mybir.dt.bfloat16
    f32 = mybir.dt.float32
    EXP = mybir.ActivationFunctionType.Exp
    COPY = mybir.ActivationFunctionType.Copy
    RADD = bass_isa.ReduceOp.add

    nc = bacc.Bacc('TRN2', target_bir_lowering=False, debug=False)

    xT = nc.dram_tensor('xT', [H, S], bf16, kind='ExternalInput')
    wqh = nc.dram_tensor('wqh', [HG, H], bf16, kind='ExternalInput')
    wkh = nc.dram_tensor('wkh', [HG, H], bf16, kind='ExternalInput')
    wvp = nc.dram_tensor('wvp', [128, KT * HG], bf16, kind='ExternalInput')
    woT = nc.dram_tensor('woT', [HG, H], bf16, kind='ExternalInput')
    cosb = nc.dram_tensor('cosb', [HD, S], bf16, kind='ExternalInput')
    s2b = nc.dram_tensor('s2b', [HD, S], bf16, kind='ExternalInput')
    maskb = nc.dram_tensor('maskb', [128, ST], f32, kind='ExternalInput')
    out = nc.dram_tensor('out', [S, H], bf16, kind='ExternalOutput')

    with tile.TileContext(nc) as tc, ExitStack() as top:
        # pool entry order is allocation-stack order; xp and pp are innermost
        # so they can be released (x tiles + projection PSUM) before the
        # o_proj-era pools open
        persist = top.enter_context(tc.tile_pool(name='persist', bufs=1))
        big = top.enter_context(tc.tile_pool(name='big', bufs=4))
        qkp = top.enter_context(tc.tile_pool(name='qkp', bufs=4))
        stg = top.enter_context(tc.tile_pool(name='stg', bufs=3))
        ep = top.enter_context(tc.tile_pool(name='ep', bufs=6))
        tpp = top.enter_context(tc.tile_pool(name='tpp', bufs=2))
        sr = top.enter_context(tc.tile_pool(name='sr', bufs=1))
        ps_cm = tc.tile_pool(name='ps_p', bufs=2, space='PSUM')
        ps_p = ps_cm.__enter__()
        pa_cm = tc.tile_pool(name='pa_p', bufs=2, space='PSUM')
        pa_p = pa_cm.__enter__()
        # third scores bank, alive only while the o_proj PSUM pool is not:
        # breaks the ps=2 exp/scores semaphore lockstep in the att0-2 cells
        ps2_cm = tc.tile_pool(name='ps2', bufs=1, space='PSUM')
        ps2_p = ps2_cm.__enter__()
        xp_cm = tc.tile_pool(name='xp', bufs=1)
        xp = xp_cm.__enter__()
        pp_cm = tc.tile_pool(name='pp', bufs=3, space='PSUM')
        pp = pp_cm.__enter__()

        # ---- persistent tiles ----
        xt = [xp.tile([128, S], bf16, tag=f'x{k}', name=f'x{k}') for k in range(KT)]
        wqt = [persist.tile([128, H], bf16, tag=f'wq{i}', name=f'wq{i}') for i in range(G)]
        wkt = [persist.tile([128, H], bf16, tag=f'wk{i}', name=f'wk{i}') for i in range(G)]
        # wv segments and (later) wo tiles share one 4-buffer region: wv is
        # dead after the V phase, wo is only needed from the o_proj region on;
        # tag rotation inserts the WAR dependency automatically.
        wvs = [big.tile([128, S], bf16, tag='seg', name=f'wv{s}') for s in range(4)]
        cos_t = persist.tile([HD, S], bf16, tag='cos', name='cos')
        s2_t = persist.tile([HD, S], bf16, tag='s2', name='s2')
        mb_t = persist.tile([128, ST], f32, tag='mb', name='mb')
        vt = [persist.tile([128, HG], bf16, tag=f'v{t}', name=f'v{t}') for t in range(ST)]
        att = [[persist.tile([128, 512], bf16, tag=f'att{h}_{c}', name=f'att{h}_{c}')
                for c in range(SC)] for h in range(G)]
        qk = {'q': {}, 'k': {}}

        # ---- DMA issue order: all q/k heads + x chunk 0 first (eight
        # chunk-0 chains cover the PE while x chunks 1-3 stream), wv, mask ----
        nc.sync.dma_start(wqt[0][:, 0:256], wqh[0:128, 0:256])
        nc.sync.dma_start(xt[0][:, 0:512], xT[0:128, 0:512])
        nc.sync.dma_start(wqt[0][:, 256:H], wqh[0:128, 256:H])
        nc.sync.dma_start(xt[1][:, 0:512], xT[128:256, 0:512])
        nc.sync.dma_start(wkt[0][:], wkh[0:128, :])
        for k in range(2, KT):
            nc.sync.dma_start(xt[k][:, 0:512], xT[k * 128:(k + 1) * 128, 0:512])
        for i in range(1, G):
            nc.sync.dma_start(wqt[i][:], wqh[i * 128:(i + 1) * 128, :])
            nc.sync.dma_start(wkt[i][:], wkh[i * 128:(i + 1) * 128, :])
        nc.sync.dma_start(cos_t[:], cosb[:])
        nc.sync.dma_start(s2_t[:], s2b[:])
        for k in range(KT):
            nc.sync.dma_start(xt[k][:, 512:S], xT[k * 128:(k + 1) * 128, 512:S])
        for s in range(4):
            nc.sync.dma_start(wvs[s][:], wvp[:, s * S:(s + 1) * S])
        nc.sync.dma_start(mb_t[:], maskb[:])

        def get_qk(nm, h):
            if h not in qk[nm]:
                qk[nm][h] = qkp.tile([128, S], bf16, tag=nm, name=f'{nm}{h}')
            return qk[nm][h]

        def proj_chunk_gen(nm, h, c, pool=None, ptag='pp'):
            """Generator: q or k projection + RoPE for head h, chunk c.
            Yields after each PE matmul (16 yields)."""
            wt = wqt[h] if nm == 'q' else wkt[h]
            dst = get_qk(nm, h)
            cs = slice(c * 512, (c + 1) * 512)
            ps = (pool or pp).tile([128, 512], f32, tag=ptag, name='pjps')
            for k in range(KT):
                nc.tensor.matmul(
                    ps[:],
                    lhsT=wt[:, k * 128:(k + 1) * 128],
                    rhs=xt[k][:, cs],
                    start=(k == 0), stop=(k == KT - 1),
                )
                yield
            # RoPE: rotated-half muls read PSUM directly (SBUF-SBUF DVE ops
            # require equal base partitions); cos mul + add run in bf16
            qraw = stg.tile([128, 512], bf16, tag='qraw', name='qraw')
            nc.scalar.activation(qraw[:], ps[:], COPY)
            t2 = stg.tile([128, 512], bf16, tag='t2', name='t2')
            nc.vector.tensor_mul(t2[0:64, :], ps[64:128, :], s2_t[0:64, cs])
            nc.vector.tensor_mul(t2[64:128, :], ps[0:64, :], s2_t[64:128, cs])
            nc.vector.tensor_mul(dst[:, cs], qraw[:], cos_t[:, cs])
            nc.vector.tensor_add(dst[:, cs], dst[:, cs], t2[:])

        def proj_head_gen(nm, h, chunks=range(SC)):
            for c in chunks:
                yield from proj_chunk_gen(nm, h, c)

        def v_chain_gen():
            """V projection: 16 s-tiles, yields after each PE matmul."""
            for si in range(ST):
                ps = pp.tile([128, HG], f32, tag='pp', name='pp')
                for k in range(KT):
                    seg, kk = divmod(k, 4)
                    nc.tensor.matmul(
                        ps[:],
                        lhsT=xt[k][:, si * 128:(si + 1) * 128],
                        rhs=wvs[seg][:, kk * HG:(kk + 1) * HG],
                        start=(k == 0), stop=(k == KT - 1),
                    )
                    yield
                nc.scalar.activation(vt[si][:], ps[:], COPY)

        def run(gen):
            for _ in gen:
                pass

        pending_norm = []

        def flush_norms():
            while pending_norm:
                pending_norm.pop(0)()

        def att_cell(h, c, partner, nsteps, oproj_emit=None, fast_tail=False):
            """Attention cell (h, c): scoresT -> exp -> pa accumulation,
            denominator tree + allreduce, normalize. `partner` supplies
            matmuls interleaved between scores so ACT keeps up. oproj_emit:
            optional per-slot callback emitting o_proj matmuls."""
            cs = slice(c * 512, (c + 1) * 512)
            hs = slice(h * 128, (h + 1) * 128)
            qh, kh = qk['q'][h], qk['k'][h]
            pa = pa_p.tile([128, 512], f32, tag='pa', name='pa')
            es = []
            tps = []
            deep = h < 3  # att3 runs while ps2 is closed
            # att3 cell 0 is ACT-bound with short slots: borrow idle o_proj
            # banks (slots 0-11 only; q3c1 + the pre-opened chains use the
            # rest) to get the same depth-3 scores pipeline
            deep2 = h == 3 and c == 0
            for t in range(ST):
                if deep and t % 3 == 2:
                    ps = ps2_p.tile([128, 512], f32, tag='ps2', name='ps2')
                elif deep2 and t % 3 == 2 and t < 12:
                    ps = po_p.tile([128, 512], f32, tag='po', name='po')
                else:
                    ps = ps_p.tile([128, 512], f32, tag='ps', name='ps')
                nc.tensor.matmul(
                    ps[:],
                    lhsT=kh[:, t * 128:(t + 1) * 128],
                    rhs=qh[:, cs],
                    start=True, stop=True,
                )
                e = ep.tile([128, 512], bf16, tag='e', name='e')
                nc.scalar.activation(e[:], ps[:], EXP, bias=mb_t[:, t:t + 1], scale=1.0)
                es.append(e)
                if t == 2:
                    # previous cell's normalize goes here, behind this cell's
                    # first DVE tree adds, so its allreduce wait can't
                    # head-of-line-block the DVE queue
                    flush_norms()
                if partner is not None:
                    n = nsteps[t] if isinstance(nsteps, (list, tuple)) else nsteps
                    for _ in range(n):
                        next(partner, None)
                if oproj_emit is not None:
                    oproj_emit(t)
                if t >= 2:
                    nc.tensor.matmul(pa[:], lhsT=vt[t - 2][:, hs], rhs=es[t - 2][:],
                                     start=(t == 2), stop=False)
                # denominator tree on DVE: u_j accumulates e[4j..4j+3];
                # u1..u3 merge into the running total u0 as soon as complete
                # so the post-cell serial tail is just one add + allreduce
                if t % 4 == 1:
                    u = tpp.tile([128, 512], bf16, tag=f'u{t // 4}', name=f'u{t // 4}')
                    nc.vector.tensor_add(u[:], es[t - 1][:], es[t][:])
                    tps.append(u)
                elif t % 4 in (2, 3):
                    u = tps[t // 4]
                    nc.vector.tensor_add(u[:], u[:], es[t][:])
                    if t % 4 == 3 and t // 4 >= 1:
                        nc.vector.tensor_add(tps[0][:], tps[0][:], u[:])
            nc.tensor.matmul(pa[:], lhsT=vt[ST - 2][:, hs], rhs=es[ST - 2][:],
                             start=False, stop=False)
            nc.tensor.matmul(pa[:], lhsT=vt[ST - 1][:, hs], rhs=es[ST - 1][:],
                             start=False, stop=True)
            sumall = sr.tile([128, 512], f32, tag='sum', name='sumall')
            nc.gpsimd.partition_all_reduce(sumall[:], tps[0][:], 128, RADD)

            def _norm(pa=pa, sumall=sumall, dst=att[h][c]):
                rec = sr.tile([128, 512], bf16, tag='rec', name='rec')
                with nc.allow_low_precision(reason='bf16 softmax reciprocal within tolerance'):
                    nc.vector.reciprocal(rec[:], sumall[:])
                nc.vector.tensor_mul(dst[:], pa[:], rec[:])
            if fast_tail:
                _norm()
            else:
                pending_norm.append(_norm)

        # ---------------- pipeline ----------------
        # startup: all eight chunk-0 q/k chains run on resident data while x
        # chunks 1-3 stream in; the late chains borrow the still-idle
        # attention PSUM pools so RoPE lag (waiting on cos/sin DMA) can't
        # exhaust pp and stall the PE
        startup_pools = ([(pp, 'pp')] * 3 + [(pa_p, 'pa')] * 2 +
                         [(ps_p, 'ps')] * 2 + [(ps2_p, 'ps2')])
        for hh in range(G):
            pq, tq = startup_pools[2 * hh]
            pk, tk = startup_pools[2 * hh + 1]
            run(proj_chunk_gen('q', hh, 0, pool=pq, ptag=tq))
            run(proj_chunk_gen('k', hh, 0, pool=pk, ptag=tk))
        for c in range(1, SC):
            for hh in range(2):
                run(proj_chunk_gen('q', hh, c))
                if not (hh == 1 and c == SC - 1):
                    run(proj_chunk_gen('k', hh, c))
        run(v_chain_gen())
        # wo reuses the wv buffers (WAR dep on the V chains, lands mid-kernel)
        wo_t = [big.tile([128, S], bf16, tag='seg', name=f'wo{j}') for j in range(G)]
        for j in range(G):
            nc.sync.dma_start(wo_t[j][:], woT[j * 128:(j + 1) * 128, :])

        def chained(*gens):
            for g in gens:
                yield from g

        def diluted(gen, skip_every):
            """Yield-through wrapper that idles every skip_every-th slot."""
            i = 0
            while True:
                i += 1
                if i % skip_every != 0:
                    try:
                        next(gen)
                    except StopIteration:
                        return
                yield

        # partner chains: exactly one 16-matmul projection chain per cell
        # slot keeps the PE issue rate per exp above ACT's ~570ns service
        # time with no PE idle (small gaps also reset the PE p-state ramp).
        # Ordering honors: k1c3 before att1; q2/k2 before att2; q3c2/c3
        # before their att3 cells; k3 complete before att3 cell 0.
        partner = chained(
            proj_chunk_gen('k', 1, 3),
            proj_head_gen('q', 2, range(1, SC)),
            proj_head_gen('k', 2, range(1, SC)),
            proj_chunk_gen('k', 3, 1),
            proj_chunk_gen('k', 3, 2), proj_chunk_gen('k', 3, 3),
            proj_chunk_gen('q', 3, 2), proj_chunk_gen('q', 3, 3),
        )
        for hh in range(3):
            for c in range(SC):
                att_cell(hh, c, partner, 1)
        run(partner)

        # free x tiles, projection PSUM, and the third scores bank before
        # the o_proj pools open
        pp_cm.__exit__(None, None, None)
        xp_cm.__exit__(None, None, None)
        ps2_cm.__exit__(None, None, None)
        po_cm = tc.tile_pool(name='po_p', bufs=4, space='PSUM')
        po_p = po_cm.__enter__()
        so_cm = tc.tile_pool(name='so_p', bufs=8)
        so_p = so_cm.__enter__()

        def oproj_open(cq, j, nh):
            """Start o_proj chain j of chunk-group cq: accumulate heads
            0..nh-1 into a fresh po tile."""
            si = cq * 4 + j // 4
            off = (si % 4) * 128
            ns = slice((j % 4) * 512, (j % 4 + 1) * 512)
            po = po_p.tile([128, 512], f32, tag='po', name='po')
            for hh in range(nh):
                nc.tensor.matmul(
                    po[:],
                    lhsT=att[hh][cq][:, off:off + 128],
                    rhs=wo_t[hh][:, ns],
                    start=(hh == 0), stop=(hh == G - 1),
                )
            return po

        def oproj_close(po, cq, j):
            """Finish chain j: h=3 matmul, then two half-width bf16 copies
            in parallel on ACT and DVE (halves the po-bank hold time), DMA."""
            si = cq * 4 + j // 4
            off = (si % 4) * 128
            ns = slice((j % 4) * 512, (j % 4 + 1) * 512)
            nc.tensor.matmul(po[:], lhsT=att[3][cq][:, off:off + 128],
                             rhs=wo_t[3][:, ns], start=False, stop=True)
            so = so_p.tile([128, 512], bf16, tag='so', name='so')
            if j % 2 == 0:
                nc.scalar.activation(so[:], po[:], COPY)
            else:
                nc.vector.tensor_copy(so[:], po[:])
            nc.sync.dma_start(out[si * 128:(si + 1) * 128, ns], so[:])

        def oproj_chain(cq, j):
            oproj_close(oproj_open(cq, j, 3), cq, j)

        # att(3) cells carry o_proj chains from a queue: group c-1 is
        # enqueued at cell c; a chain is popped each slot, but a cell's own
        # group only from slot 3 on (so att[3][c-1] normalization has landed
        # before the h=3 matmul). Spillover chains fill the next cell's
        # early slots. q3c1 (needed by cell 1's scores) interleaves into
        # cell 0 via the o_proj PSUM pool.
        preopened = {}

        def pre_emit(t):
            # fill the ACT-bound back slots of cell (3,0) with the first
            # group's h0-h2 matmuls (att[0..2][0] are ready); their h=3
            # closes run in cell (3,1) once att[3][0] is normalized
            if t >= 12:
                j = t - 12
                preopened[(0, j)] = oproj_open(0, j, 3)
        q3c1 = proj_chunk_gen('q', 3, 1, pool=po_p, ptag='po')
        att_cell(3, 0, q3c1, [2, 2, 2, 2, 1, 1, 1, 1, 1, 1, 1, 1, 0, 0, 0, 0],
                 oproj_emit=pre_emit)
        run(q3c1)
        oq = []
        for c in range(1, SC):
            for j in range(16):
                oq.append((c - 1, j, c))

            def emit(t, cur=c):
                if oq and (t >= 3 or oq[0][2] < cur):
                    cq, j, _ = oq.pop(0)
                    po = preopened.pop((cq, j), None)
                    if po is not None:
                        oproj_close(po, cq, j)
                    else:
                        oproj_chain(cq, j)
            att_cell(3, c, None, 0, oproj_emit=emit, fast_tail=(c == SC - 1))
        # drain older spillover chains, then software-pipeline the last
        # group: heads 0-2 of upcoming chains run while the previous chain
        # waits on the final att[3][3] normalization
        flush_norms()
        work = [(cq, j) for cq, j, _ in oq] + [(SC - 1, j) for j in range(16)]
        DEPTH = 4
        pending = []
        for cq, j in work[:DEPTH]:
            pending.append((oproj_open(cq, j, 3), cq, j))
        for idx, (cq, j) in enumerate(work):
            po, _, _ = pending.pop(0)
            oproj_close(po, cq, j)
            nxt = idx + DEPTH
            if nxt < len(work):
                ncq, nj = work[nxt]
                pending.append((oproj_open(ncq, nj, 3), ncq, nj))
        so_cm.__exit__(None, None, None)
        po_cm.__exit__(None, None, None)
        pa_cm.__exit__(None, None, None)
        ps_cm.__exit__(None, None, None)
    nc.finalize()
    return nc


def _get_nc():
    if 'nc' not in _NC_CACHE:
        _NC_CACHE['nc'] = _build_nc()
    return _NC_CACHE['nc']


def _prep_in_maps(hidden_states, attention_mask, wq, wk, wv, wo):
    bf = ml_dtypes.bfloat16
    inv = 1.0 / (10000.0 ** (np.arange(0, HD, 2, dtype=np.float32) / np.float32(HD)))
    t = np.arange(S, dtype=np.float32)
    freqs = np.outer(t, inv).astype(np.float32)          # [S, 64]
    emb = np.concatenate([freqs, freqs], axis=1)         # [S, 128]
    cosT = np.ascontiguousarray(np.cos(emb).T).astype(bf)   # [128, S]
    s2T = np.sin(emb).T.astype(np.float32)
    s2T[:64] *= np.float32(-1.0)
    s2T = np.ascontiguousarray(s2T).astype(bf)
    scale = np.float32(1.0) / np.sqrt(np.float32(HD))

    hs = np.asarray(hidden_states, dtype=np.float32)
    mask = np.asarray(attention_mask)
    wq = np.asarray(wq, dtype=np.float32)
    wk = np.asarray(wk, dtype=np.float32)
    wv = np.asarray(wv, dtype=np.float32)
    wo = np.asarray(wo, dtype=np.float32)

    def pack_heads(wT):
        # wT [H, HG] -> [HG, H]: row block i*128+p, col k*128+cc =
        # wT[k*128+p, i*128+cc]
        blocks = []
        for i in range(G):
            A = wT[:, i * 128:(i + 1) * 128]                   # [H, 128]
            blocks.append(A.reshape(KT, 128, 128).transpose(1, 0, 2).reshape(128, H))
        return np.ascontiguousarray(np.concatenate(blocks, axis=0))  # [HG, H]

    in_maps = []
    for core in range(NCORES):
        b, g = divmod(core, G)
        cols = slice(g * HG, (g + 1) * HG)
        xTc = np.ascontiguousarray(hs[b].T).astype(bf)
        wqTc = (wq[cols, :] * scale).T.astype(np.float32)      # [H, HG]
        wkTc = wk[cols, :].T.astype(np.float32)
        wvTc = wv[cols, :].T.astype(np.float32)                # [H, HG]
        wqhc = pack_heads(wqTc).astype(bf)
        wkhc = pack_heads(wkTc).astype(bf)
        wvpc = np.ascontiguousarray(
            wvTc.reshape(KT, 128, HG).transpose(1, 0, 2).reshape(128, KT * HG)
        ).astype(bf)
        woTc = np.ascontiguousarray(wo[:, cols].T).astype(bf)  # [HG, H]
        mb = np.where(mask[b] == 0, np.float32(-1e30), np.float32(0.0))
        mbc = np.ascontiguousarray(mb.astype(np.float32).reshape(ST, 128).T)
        in_maps.append({
            'xT': xTc, 'wqh': wqhc, 'wkh': wkhc, 'wvp': wvpc, 'woT': woTc,
            'cosb': cosT, 's2b': s2T, 'maskb': mbc,
        })
    return in_maps


def kernel(hidden_states, attention_mask, wq, wk, wv, wo):
    _ensure_path()
    from concourse import bass_utils
    nc = _get_nc()
    in_maps = _prep_in_maps(hidden_states, attention_mask, wq, wk, wv, wo)
    res = bass_utils.run_bass_kernel_spmd(nc, in_maps, core_ids=list(range(NCORES)))
    outs = [r['out'] for r in res.results]
    full = np.empty((B, S, H), np.float32)
    for b in range(B):
        acc = outs[G * b].astype(np.float32)
        for g in range(1, G):
            acc = acc + outs[G * b + g]
        full[b] = acc
    return full


if __name__ == '__main__':
    rng = np.random.default_rng(0)
    ins = {
        'hidden_states': rng.standard_normal((B, S, H), dtype=np.float32),
        'attention_mask': np.ones((B, S), np.int32),
        'wq': rng.standard_normal((H, H), dtype=np.float32) / np.sqrt(H),
        'wk': rng.standard_normal((H, H), dtype=np.float32) / np.sqrt(H),
        'wv': rng.standard_normal((H, H), dtype=np.float32) / np.sqrt(H),
        'wo': rng.standard_normal((H, H), dtype=np.float32) / np.sqrt(H),
    }
    out = kernel(**ins)
    print('out', out.shape, out.dtype, float(np.abs(out).mean()))
